# revision 1
# baseline (speedup 1.0000x reference)
"""Trainium2 Bass kernel for CRFDecoder.fit (sum reduction).

Math (scaled forward algorithm, probability space, no padding mask):
  q_0[j,b]  = exp(start[j] + em[0,b,j])                     (bf16, tag-major [256,16])
  q_t       = (expT.T @ q_{t-1}) * exp(em_t - LAM)          (PE bf16 matmuls, fp32 PSUM)
  z_t[b]    = sum_j q_t[j,b] * expEnd[j]                    (PE, persistent PSUM bank [16,512])
  logZ_b    = ln(z_{len_b-1}[b]) + LAM*(len_b-1)
  score_b   = gathers of em / transition / start / end via indirect DMA (padded -> zero slot)
  out       = sum_b (logZ_b - score_b)                      (per-core partial; host sums 8)

Sharding: data-parallel over batch: core c handles batch columns [16c, 16c+16).
Tag dim 256 is split as j = h*128 + j_lo (h in {0,1}); state tiles are [128, 2, 16].
Emission is host-rearranged per core to [j_lo, t, h, b] bf16 so every DMA is contiguous.
"""

import os

import numpy as np
import ml_dtypes

SLN, BSZ, TAG = 512, 128, 256
NCORES = 8
B = BSZ // NCORES          # 16 per-core batch
P = 128                    # partitions
H = TAG // P               # 2 tag halves
LAM = float(np.log(TAG) + 0.5)
EM_N = P * SLN * H * B     # flattened emission elements per core
TSE_T = TAG * TAG          # transition table size
TSE_S = TSE_T              # start offset base
TSE_E = TSE_T + TAG        # end offset base
TSE_Z = TSE_T + 2 * TAG    # zero slot base
TSE_N = TSE_T + 2 * TAG + B

bf16 = ml_dtypes.bfloat16

_CACHE: dict = {}


def _build_bass():
    import concourse.bacc as bacc
    import concourse.tile as tile
    from concourse import mybir
    import concourse.bass as bass

    nc = bacc.Bacc(
        "TRN2",
        target_bir_lowering=False,
        debug=False,
        enable_asserts=False,
        num_devices=NCORES,
    )
    f32 = mybir.dt.float32
    bft = mybir.dt.bfloat16
    i32 = mybir.dt.int32

    em_h = nc.dram_tensor("em", [EM_N], bft, kind="ExternalInput")
    expT_h = nc.dram_tensor("expT", [H, H, P, P], bft, kind="ExternalInput")
    startT_h = nc.dram_tensor("startT", [P, H], f32, kind="ExternalInput")
    expEnd_h = nc.dram_tensor("expEnd", [P, H], bft, kind="ExternalInput")
    lamlen_h = nc.dram_tensor("lamlen", [B, 1], f32, kind="ExternalInput")
    onehot_h = nc.dram_tensor("onehot", [B, SLN], f32, kind="ExternalInput")
    emv_h = nc.dram_tensor("emv", [B, SLN], f32, kind="ExternalInput")
    tv_h = nc.dram_tensor("tv", [B, SLN + 1], f32, kind="ExternalInput")
    emm_h = nc.dram_tensor("emm", [B, SLN], f32, kind="ExternalInput")
    tm_h = nc.dram_tensor("tm", [B, SLN + 1], f32, kind="ExternalInput")
    out_h = nc.dram_tensor("out", [1, 1], f32, kind="ExternalOutput")

    em_view = em_h.ap()[:EM_N].rearrange(
        "(p t h b) -> p t h b", p=P, t=SLN, h=H, b=B
    )

    NSB = 8                 # emission superblocks
    SBL = SLN // NSB        # 64 steps per superblock

    from contextlib import ExitStack

    with tile.TileContext(nc) as tc, ExitStack() as es:
        persist = es.enter_context(tc.tile_pool(name="persist", bufs=1))

        def st(shape, dtype, name):
            return persist.tile(shape, dtype, name=name, tag=name)

        # ---- constants ----
        expT_sb = st([P, H, H, P], bft, name="expT_sb")   # (i_lo, k, h, j_lo)
        for k in range(H):
            for h in range(H):
                nc.sync.dma_start(out=expT_sb[:, k, h, :], in_=expT_h.ap()[k, h, :, :])
        startT_sb = st([P, H], f32, name="startT_sb")
        nc.sync.dma_start(out=startT_sb, in_=startT_h.ap())
        expEnd_sb = st([P, H], bft, name="expEnd_sb")
        nc.sync.dma_start(out=expEnd_sb, in_=expEnd_h.ap())
        lamlen_sb = st([B, 1], f32, name="lamlen_sb")
        nc.sync.dma_start(out=lamlen_sb, in_=lamlen_h.ap())
        onehot_sb = st([B, SLN], f32, name="onehot_sb")
        nc.sync.dma_start(out=onehot_sb, in_=onehot_h.ap())
        emv_sb = st([B, SLN], f32, name="emv_sb")
        nc.sync.dma_start(out=emv_sb, in_=emv_h.ap())
        tv_sb = st([B, SLN + 1], f32, name="tv_sb")
        nc.sync.dma_start(out=tv_sb, in_=tv_h.ap())
        emm_sb = st([B, SLN], f32, name="emm_sb")
        nc.sync.dma_start(out=emm_sb, in_=emm_h.ap())
        tm_sb = st([B, SLN + 1], f32, name="tm_sb")
        nc.sync.dma_start(out=tm_sb, in_=tm_h.ap())
        ones_sb = st([B, 1], f32, name="ones_sb")
        nc.vector.memset(ones_sb, 1.0)
        neglam_sb = st([P, 1], f32, name="neglam_sb")
        nc.vector.memset(neglam_sb, -LAM)

        # ---- emission load + exp (prefetched per superblock) ----
        em_t = []
        expem_t = []
        emp = es.enter_context(tc.tile_pool(name="emp", bufs=NSB))
        exq = es.enter_context(tc.tile_pool(name="exp", bufs=NSB))
        if True:
            for i in range(NSB):
                emt = emp.tile([P, SBL, H, B], bft, tag="emt")
                nc.sync.dma_start(
                    out=emt, in_=em_view[:, i * SBL : (i + 1) * SBL, :, :]
                )
                em_t.append(emt)
                xt = exq.tile([P, SBL, H, B], bft, tag="xt")
                nc.scalar.activation(
                    xt,
                    emt,
                    mybir.ActivationFunctionType.Exp,
                    bias=neglam_sb[:],
                    scale=1.0,
                )
                expem_t.append(xt)

            qp = es.enter_context(tc.tile_pool(name="qp", bufs=3))
            up = es.enter_context(tc.tile_pool(name="up", bufs=2, space="PSUM"))
            zp = es.enter_context(tc.tile_pool(name="zp", bufs=1, space="PSUM"))
            if True:
                # q0 = exp(em0 + start)
                q = qp.tile([P, H, B], bft, tag="q")
                for h in range(H):
                    nc.scalar.activation(
                        q[:, h, :],
                        em_t[0][:, 0, h, :],
                        mybir.ActivationFunctionType.Exp,
                        bias=startT_sb[:, h : h + 1],
                        scale=1.0,
                    )

                z_ps = zp.tile([B, SLN], mybir.dt.float32)

                NSTEPS = int(os.environ.get("CRF_STEPS", SLN))
                for t in range(1, NSTEPS):
                    sb, col = divmod(t, SBL)
                    u = up.tile([P, H, B], mybir.dt.float32, tag="u")
                    for h in range(H):
                        for k in range(H):
                            nc.tensor.matmul(
                                u[:, h, :],
                                expT_sb[:, k, h, :],
                                q[:, k, :],
                                start=(k == 0),
                                stop=(k == H - 1),
                            )
                    # z for the PREVIOUS state (q is q_{t-1} here); t-1 == 0 is
                    # never selected (len >= 2) but write it anyway so the PSUM
                    # bank holds no garbage (NaN * 0.0 would poison the reduce).
                    if True:
                        for k in range(H):
                            nc.tensor.matmul(
                                z_ps[:, t - 1 : t],
                                q[:, k, :],
                                expEnd_sb[:, k : k + 1],
                                start=(k == 0),
                                stop=(k == H - 1),
                            )
                    qn = qp.tile([P, H, B], bft, tag="q")
                    nc.vector.tensor_mul(qn, u, expem_t[sb][:, col, :, :])
                    q = qn

                # z for the final state t = SLN-1
                for k in range(H):
                    nc.tensor.matmul(
                        z_ps[:, SLN - 1 : SLN],
                        q[:, k, :],
                        expEnd_sb[:, k : k + 1],
                        start=(k == 0),
                        stop=(k == H - 1),
                    )

                # ---- finalization ----
                z_sb = st([B, SLN], mybir.dt.float32, name="z_sb")
                nc.vector.tensor_copy(z_sb, z_ps)

        prod_sb = st([B, SLN], mybir.dt.float32, name="prod_sb")
        z_sel = st([B, 1], mybir.dt.float32, name="z_sel")
        nc.vector.tensor_mul(prod_sb, z_sb, onehot_sb)
        nc.vector.reduce_sum(z_sel, prod_sb, axis=mybir.AxisListType.X)
        logz = st([B, 1], mybir.dt.float32, name="logz")
        nc.scalar.activation(logz, z_sel, mybir.ActivationFunctionType.Ln)
        logz2 = st([B, 1], mybir.dt.float32, name="logz2")
        nc.vector.tensor_add(logz2, logz, lamlen_sb)

        emprod = st([B, SLN], mybir.dt.float32, name="emprod")
        em_part = st([B, 1], mybir.dt.float32, name="em_part")
        nc.vector.tensor_mul(emprod, emv_sb, emm_sb)
        nc.vector.reduce_sum(em_part, emprod, axis=mybir.AxisListType.X)
        tprod = st([B, SLN + 1], mybir.dt.float32, name="tprod")
        t_part = st([B, 1], mybir.dt.float32, name="t_part")
        nc.vector.tensor_mul(tprod, tv_sb, tm_sb)
        nc.vector.reduce_sum(t_part, tprod, axis=mybir.AxisListType.X)

        score = st([B, 1], mybir.dt.float32, name="score")
        nc.vector.tensor_add(score, em_part, t_part)
        res = st([B, 1], mybir.dt.float32, name="res")
        nc.vector.tensor_sub(res, logz2, score)

        tp = es.enter_context(tc.tile_pool(name="tp", bufs=1, space="PSUM"))
        tot_ps = tp.tile([1, 1], mybir.dt.float32)
        nc.tensor.matmul(tot_ps, res, ones_sb, start=True, stop=True)
        tot_sb = st([1, 1], mybir.dt.float32, name="tot_sb")
        nc.vector.tensor_copy(tot_sb, tot_ps)
        nc.sync.dma_start(out=out_h.ap(), in_=tot_sb)

    nc.compile()
    return nc


def _prep_inputs(emission, length, target, transition, start_transition, end_transition):
    """Host-side sharding/layout prep. Returns list of per-core input dicts."""
    emission = np.asarray(emission, np.float32)
    length = np.asarray(length).astype(np.int64)
    target = np.asarray(target).astype(np.int64)
    T = np.asarray(transition, np.float32)
    startT = np.asarray(start_transition, np.float32)
    endT = np.asarray(end_transition, np.float32)

    expT_full = np.exp(T, dtype=np.float32)
    expT_arr = np.zeros((H, H, P, P), bf16)
    for k in range(H):
        for h in range(H):
            expT_arr[k, h] = expT_full[k * P : (k + 1) * P, h * P : (h + 1) * P].astype(
                bf16
            )
    startT_arr = np.ascontiguousarray(
        startT.reshape(H, P).T, dtype=np.float32
    )  # [j_lo, h]
    expEnd_arr = np.ascontiguousarray(np.exp(endT).reshape(H, P).T).astype(bf16)

    in_maps = []
    for c in range(NCORES):
        bs = slice(c * B, (c + 1) * B)
        emc = emission[:, bs, :]                    # [512,16,256]
        lenc = length[bs]                           # [16]
        tgt = target[:, bs]                         # [512,16]

        # [j_lo, t, h, b] layout, contiguous (h,b) runs of 64B
        em_r = np.transpose(
            emc.reshape(SLN, B, H, P), (3, 0, 2, 1)
        )  # [j_lo, t, h, b]
        em_arr = np.ascontiguousarray(em_r).astype(bf16).ravel()

        tt = np.arange(SLN)[:, None]
        pad = tt >= lenc[None, :]                   # [512,16]
        bb = np.arange(B)

        # score tables: host does PURE INDEXING; all arithmetic on device
        emv = np.take_along_axis(emc, tgt[:, :, None], axis=2)[:, :, 0].T  # [16,512]
        emv = np.ascontiguousarray(emv, np.float32)
        emm = np.ascontiguousarray((~pad).T, np.float32)          # [16,512]
        tv = np.zeros((B, SLN + 1), np.float32)
        tv[:, 0] = startT[tgt[0]]
        tv[:, 1:SLN] = T[tgt[:-1], tgt[1:]].T
        tv[:, SLN] = endT[tgt[lenc - 1, bb]]
        tm = np.ones((B, SLN + 1), np.float32)
        tm[:, 1:SLN] = (~pad[1:]).T

        onehot = np.zeros((B, SLN), np.float32)
        onehot[bb, lenc - 1] = 1.0
        lamlen = (LAM * (lenc - 1)).astype(np.float32).reshape(B, 1)

        in_maps.append(
            dict(
                em=em_arr,
                expT=expT_arr,
                startT=startT_arr,
                expEnd=expEnd_arr,
                lamlen=lamlen,
                onehot=onehot,
                emv=emv,
                tv=tv,
                emm=emm,
                tm=tm,
            )
        )
    return in_maps


def kernel(
    emission,
    length,
    padding_mask,
    target,
    transition,
    start_transition,
    end_transition,
):
    from concourse import bass_utils

    in_maps = _prep_inputs(
        emission, length, target, transition, start_transition, end_transition
    )
    if "nc" not in _CACHE:
        _CACHE["nc"] = _build_bass()
    nc = _CACHE["nc"]
    res = bass_utils.run_bass_kernel_spmd(
        nc, in_maps, core_ids=list(range(NCORES))
    )
    total = np.float32(0.0)
    for c in range(NCORES):
        total += np.float32(res.results[c]["out"].reshape(-1)[0])
    return np.asarray(total, dtype=np.float32)

